# revision 43
# baseline (speedup 1.0000x reference)
"""Trainium2 Bass kernel for nn_Attention_85710367359111.

Full (unsharded) numpy inputs in, full output out. Tensor-parallel over
heads (16 heads / 8 cores = 2 heads per core), on-device AllToAll to
re-shard from head-parallel to token-parallel before the output
projection, host-side concat of the 8 token blocks.

Key optimizations over the fp32 baseline (~440us/rep -> ~295us/rep):
- fp16 matmul operands everywhere (PE 1-pass vs fp32 4-pass), bf16 for
  exp outputs (range safety; max score ~19 overflows fp16 exp).
- The ACT (scalar) engine is the critical resource: exp of all S^2
  scores is ~147us minimum. Everything else is arranged around keeping
  its queue saturated: a flat two-stage software pipeline over all
  (batch, qchunk, kt) score groups -- while ACT runs exp(g), the PE
  runs o(g-1) and scores(g+1); no pipeline resets at qc boundaries.
- Batch-1 qkv/rms/rope/transpose work is issued in paced substeps
  interleaved into batch-0's attention loop, hiding under the ACT wall
  (2 PSUM overlay banks; scores 4 banks double-buffered + o-accum 2).
- Softmax normalization happens after the AllToAll: the denominator
  RECIPROCALS ride the collective as row 64/129 of each 130-row block,
  so phase D is one broadcast matmul + multiply per source core.
- exp computes exp(s - ln1024) and o is shipped scaled by 1/64
  (compensated by 64/d) to stay inside fp16 range end to end.
- Dummy matmuls bridge the AllToAll window so the PE HAM clock gate
  does not drop to the cold 4/8 rate before the output projection.
- Large merged DMAs (one per weight tensor / x strip) keep the Sync
  queue short; the first x strip is issued before cos/sin so phase A
  starts immediately.

Per-core pipeline (feature-major [feat_on_partitions, tokens]):
  A  qkv^T = w_qkv_slice @ x^T            (PE, fp16 -> fp32 PSUM)
  B  RMS stats via selector-matmul -> reciprocal(DVE) -> sqrt(ACT),
     scale via rank-2 broadcast matmul, RoPE via +-1 permutation
     matmul + cos/sin elementwise (DVE)
  B' v^T -> token-major via PE transposes (with ones columns)
  C  scores^T = k^T.T @ q^T per head (row-tiled, both heads concurrent
     on PE) -> exp (ACT) -> o_aug = [v|1].T @ ex accumulated over kt
     in PSUM -> fp16 -> DRAM
  D  AllToAll of [o_unnorm/64; 64/denom] blocks, broadcast matmul +
     multiply normalize, out^T = w_out^T.T @ gathered (row-parallel,
     no all-reduce needed)
"""

import math
import os

import numpy as np

import concourse.bacc as bacc
import concourse.bass as bass
import concourse.tile as tile
from concourse import mybir
from concourse.bass_utils import run_bass_kernel_spmd

# ---------------------------------------------------------------- config

B, S, D, H, E = 2, 2048, 1024, 16, 64
NC = 8                      # cores
HPC = H // NC               # heads per core = 2
DL = HPC * E                # local d slice = 128
T = B * S                   # total tokens
TB = T // NC                # tokens per A2A block = 512
KT = S // 128               # k tiles per batch = 16
NTT = T // 128              # total tok tiles = 32
QC = 512                    # q chunk
NQC = S // QC               # q chunks per batch = 4
CH = 512                    # phase A/B token chunk
NCH = T // CH               # chunks total = 8
BLK = 2 * (E + 1)           # osh rows per token block = 130
EPS = float(np.finfo(np.float32).eps)
LNC = float(math.log(1024.0))

f16 = mybir.dt.float16
bf16 = mybir.dt.bfloat16
f32 = mybir.dt.float32

# kt group pattern per qc: strictly alternating PSUM tags (big slot holds
# 1-2 kt, small slot 1 kt) so consecutive exp groups never share a score
# tile -- including across qc boundaries (12 groups, b/s/b/s/...)
KT_GROUPS = [(0, 2, "b"), (2, 1, "s"), (3, 2, "b"), (5, 1, "s"),
             (6, 2, "b"), (8, 1, "s"), (9, 2, "b"), (11, 1, "s"),
             (12, 1, "b"), (13, 1, "s"), (14, 1, "b"), (15, 1, "s")]
assert sum(g[1] for g in KT_GROUPS) == KT


def build_nc(s=S, reps=1):
    assert s == S, "kernel hardcoded for S=2048"
    nc = bacc.Bacc("TRN2", target_bir_lowering=False, debug=False, num_devices=NC)

    # ------------- DRAM I/O
    xT_d = nc.dram_tensor("xT", [D, T], f16, kind="ExternalInput")
    wq_d = nc.dram_tensor("wqT", [D, DL], f16, kind="ExternalInput")
    wk_d = nc.dram_tensor("wkT", [D, DL], f16, kind="ExternalInput")
    wv_d = nc.dram_tensor("wvT", [D, DL], f16, kind="ExternalInput")
    wo_d = nc.dram_tensor("woT", [D, D], f16, kind="ExternalInput")
    cos_d = nc.dram_tensor("cosT", [128, T], f16, kind="ExternalInput")
    sin_d = nc.dram_tensor("sinT", [128, T], f16, kind="ExternalInput")
    sel_d = nc.dram_tensor("sel", [128, 2], f16, kind="ExternalInput")
    wsel_d = nc.dram_tensor("wsel", [2, 128], f16, kind="ExternalInput")
    perm_d = nc.dram_tensor("permT", [128, 128], f16, kind="ExternalInput")
    iden_d = nc.dram_tensor("iden", [128, 128], f16, kind="ExternalInput")
    ones2_d = nc.dram_tensor("ones2", [2, 128], f16, kind="ExternalInput")
    onec_d = nc.dram_tensor("onecol", [128, NTT], f16, kind="ExternalInput")
    nl64_d = nc.dram_tensor("nln64", [128, 1], f32, kind="ExternalInput")

    osh_d = nc.dram_tensor("o_shard", [NC * BLK, TB], f16)
    oga_d = nc.dram_tensor("o_gath", [NC * BLK, TB], f16)
    out_d = nc.dram_tensor("out_t", [D, TB], f32, kind="ExternalOutput")
    DBG = bool(os.environ.get("KDEBUG"))
    if DBG:
        dbg = {}
        for nm, shp, dt_ in [("dbg_q01", [128, T], f32), ("dbg_qhat", [128, T], f16),
                             ("dbg_khat", [128, T], f16), ("dbg_v01", [128, T], f16),
                             ("dbg_vtok", [128, NTT * BLK], f16),
                             ("dbg_osh", [NC * BLK, TB], f16),
                             ("dbg_oga", [NC * BLK, TB], f16),
                             ("dbg_gn", [128, 8 * TB], f16),
                             ("dbg_sc", [128, 4 * QC], f32),
                             ("dbg_ex", [128, 4 * QC], bf16)]:
            dbg[nm] = nc.dram_tensor(nm, shp, dt_, kind="ExternalOutput")

    xT_v = xT_d.ap().rearrange("(dt p) t -> dt p t", p=128)     # [8,128,T]
    wo_v = wo_d.ap().rearrange("(dt p) o -> dt p o", p=128)     # [8,128,D]

    from contextlib import ExitStack

    with tile.TileContext(nc) as tc, ExitStack() as ctx:
      if True:
          if True:
            pers = ctx.enter_context(tc.tile_pool(name="pers", bufs=1))
            qhat = pers.tile([128, T], f16, tag="qhat")
            khat = pers.tile([128, T], f16, tag="khat")
            vtok = pers.tile([128, NTT, BLK], f16, tag="vtok")
            wq_s = pers.tile([128, D], f16, tag="wq")
            wk_s = pers.tile([128, D], f16, tag="wk")
            wv_s = pers.tile([128, D], f16, tag="wv")
            wo_s = pers.tile([128, 8, D], f16, tag="wo")
            sel_s = pers.tile([128, 2], f16, tag="sel")
            wsel_s = pers.tile([2, 128], f16, tag="wsel")
            perm_s = pers.tile([128, 128], f16, tag="perm")
            iden_s = pers.tile([128, 128], f16, tag="iden")
            ones2_s = pers.tile([2, 128], f16, tag="ones2")
            nl64_s = pers.tile([128, 1], f32, tag="nl64")
            q01x = [pers.tile([128, S], f32, tag=f"q01_{bb}", name=f"q01_{bb}")
                    for bb in range(2)]
            k01x = [pers.tile([128, S], f32, tag=f"k01_{bb}", name=f"k01_{bb}")
                    for bb in range(2)]
            v01x = [pers.tile([128, S], f16, tag=f"v01_{bb}", name=f"v01_{bb}")
                    for bb in range(2)]

            for w_s, w_d in ((wq_s, wq_d), (wk_s, wk_d), (wv_s, wv_d)):
                nc.sync.dma_start(
                    w_s[:].rearrange("p (dt c) -> p dt c", dt=8),
                    w_d.ap().rearrange("(dt p) c -> p dt c", p=128))
            nc.sync.dma_start(sel_s[:], sel_d.ap())
            nc.sync.dma_start(wsel_s[:], wsel_d.ap())
            nc.sync.dma_start(perm_s[:], perm_d.ap())
            nc.sync.dma_start(iden_s[:], iden_d.ap())
            nc.sync.dma_start(ones2_s[:], ones2_d.ap())
            nc.sync.dma_start(nl64_s[:], nl64_d.ap())
            # ones columns of vtok (col 64 for head0, col 129 for head1)
            nc.sync.dma_start(vtok[:, :, E], onec_d.ap())
            nc.sync.dma_start(vtok[:, :, 2 * E + 1], onec_d.ap())

            cos_t = pers.tile([128, T], f16, tag="cos")
            sin_t = pers.tile([128, T], f16, tag="sin")

            # ---------------- phases A+B+B', chunk-pipelined
            with tc.tile_pool(name="xt", bufs=2) as xtp, \
                 tc.tile_pool(name="sq", bufs=3) as sqp, \
                 tc.tile_pool(name="st", bufs=3) as stp, \
                 tc.tile_pool(name="psa", bufs=2, space="PSUM") as psa, \
                 tc.tile_pool(name="pst", bufs=1, space="PSUM") as pst, \
                 tc.tile_pool(name="pss", bufs=2, space="PSUM") as pss, \
                 tc.tile_pool(name="psy", bufs=1, space="PSUM") as psy, \
                 tc.tile_pool(name="psv", bufs=1, space="PSUM") as psv, \
                 tc.tile_pool(name="tmp", bufs=3) as tmpp:
                q01, k01, v01 = q01x[0], k01x[0], v01x[0]

                def load_strip(si):
                    xs = xtp.tile([128, 8, 2 * CH], f16, tag="xs")
                    nc.sync.dma_start(
                        xs[:],
                        xT_v[:, :, bass.ts(si, 2 * CH)].rearrange("dt p t -> p dt t"))
                    return xs

                # first x strip before cos/sin so phase A starts ASAP
                xs_cur = load_strip(0)
                nc.sync.dma_start(cos_t[:], cos_d.ap())
                nc.sync.dma_start(sin_t[:], sin_d.ap())

                for c in range(NCH // 2):
                    cs = bass.ts(c, CH)
                    # ---- A: qkv^T = W @ x^T; x loaded in 1024-tok strips,
                    # one DMA per strip for all 8 contraction tiles
                    if c % 2 == 0 and c > 0:
                        xs_cur = load_strip(c // 2)
                    xoff = (c % 2) * CH
                    for w_s, dest in ((wq_s, q01), (wk_s, k01), (wv_s, v01)):
                        ps = psa.tile([128, CH], f32, tag="ps")
                        for dt_i in range(8):
                            nc.tensor.matmul(
                                ps[:], w_s[:, bass.ts(dt_i, 128)],
                                xs_cur[:, dt_i, xoff:xoff + CH],
                                start=(dt_i == 0), stop=(dt_i == 7),
                            )
                        nc.scalar.copy(dest[:, cs], ps[:])

                    # ---- B: rms stats -> alpha -> scale -> rope (q,k)
                    st = pst.tile([2, 2 * CH], f32, tag="st")
                    for j, src_t in enumerate((q01, k01)):
                        sq = sqp.tile([128, CH], f16, tag="sq")
                        nc.scalar.activation(sq[:], src_t[:, cs],
                                             mybir.ActivationFunctionType.Square)
                        nc.tensor.matmul(st[:, bass.ts(j, CH)], sel_s[:], sq[:],
                                         start=True, stop=True)
                    # alpha/sqrt(8) = sqrt((1/8) * 1/mean)
                    rv = stp.tile([2, 2 * CH], f32, tag="rv")
                    nc.vector.reciprocal_approx_fast(rv[:], st[:])
                    alpha = stp.tile([2, 2 * CH], f16, tag="alpha")
                    nc.scalar.activation(alpha[:], rv[:],
                                         mybir.ActivationFunctionType.Sqrt,
                                         scale=0.125)
                    for j, (src_t, dest) in enumerate(((q01, qhat), (k01, khat))):
                        sps = pss.tile([128, CH], f32, tag="sps")
                        nc.tensor.matmul(sps[:], wsel_s[:], alpha[:, bass.ts(j, CH)],
                                         start=True, stop=True)
                        qs = tmpp.tile([128, CH], f16, tag="qs")
                        nc.vector.tensor_mul(qs[:], src_t[:, cs], sps[:])
                        yp = psy.tile([128, CH], f32, tag="yp")
                        nc.tensor.matmul(yp[:], perm_s[:], qs[:], start=True, stop=True)
                        t1 = tmpp.tile([128, CH], f16, tag="t1")
                        nc.vector.tensor_mul(t1[:], qs[:], cos_t[:, cs])
                        t2 = tmpp.tile([128, CH], f16, tag="t2")
                        nc.vector.tensor_mul(t2[:], yp[:], sin_t[:, cs])
                        nc.vector.tensor_add(dest[:, cs], t1[:], t2[:])

                    # ---- B': v -> token-major (+ ones cols already set)
                    g = c  # chunk == group of 4 token tiles
                    pt = psv.tile([128, 4, 128], f16, tag="pt")
                    for jj in range(4):
                        tt = g * 4 + jj
                        nc.tensor.transpose(pt[:, jj, :], v01[:, bass.ts(tt, 128)], iden_s[:])
                    nc.vector.tensor_copy(vtok[:, bass.ts(g, 4), 0:E], pt[:, :, 0:E])
                    nc.vector.tensor_copy(vtok[:, bass.ts(g, 4), E + 1:2 * E + 1],
                                          pt[:, :, E:2 * E])


            # wo loads late -- only phase D needs it
            nc.sync.dma_start(wo_s[:], wo_d.ap().rearrange("(dt p) o -> p dt o", p=128))

            # ------- persistent pools for C / overlay / D (all reps)
            scp = ctx.enter_context(tc.tile_pool(name="scp", bufs=2, space="PSUM"))
            oap = ctx.enter_context(tc.tile_pool(name="oap", bufs=1, space="PSUM"))
            ovap = ctx.enter_context(tc.tile_pool(name="ova", bufs=1, space="PSUM"))
            ovbp = ctx.enter_context(tc.tile_pool(name="ovb", bufs=1, space="PSUM"))
            xt2p = ctx.enter_context(tc.tile_pool(name="xt2", bufs=2))
            sq2p = ctx.enter_context(tc.tile_pool(name="sq2", bufs=2))
            st2p = ctx.enter_context(tc.tile_pool(name="st2", bufs=2))
            tm2p = ctx.enter_context(tc.tile_pool(name="tm2", bufs=2))
            expp = ctx.enter_context(tc.tile_pool(name="exp", bufs=3))
            ofp = ctx.enter_context(tc.tile_pool(name="ofp", bufs=2))
            gdp = ctx.enter_context(tc.tile_pool(name="gd", bufs=1))
            dnp = ctx.enter_context(tc.tile_pool(name="dnp", bufs=8))
            osb2p = ctx.enter_context(tc.tile_pool(name="osb2", bufs=2))
            ga = osh_d.ap() if os.environ.get("KNO_CC") else oga_d.ap()
            ga3 = ga.rearrange("(s r) t -> s r t", r=BLK)   # [8, 130, TB]
            xs2 = {}
            oa_cur = {}

            def mk_steps(bb, lc):
                c = bb * (NCH // 2) + lc    # global chunk index
                cs = bass.ts(c, CH)
                csl = bass.ts(lc, CH)
                xoff = (lc % 2) * CH
                q01b, k01b, v01b = q01x[bb], k01x[bb], v01x[bb]
                stqk = st2p.tile([2, 2 * CH], f32, tag="stqk", name="stqk")
                alph = st2p.tile([2, 2 * CH], f16, tag="alph", name="alph")
                sts = []

                def s_x():
                    if lc % 2 == 0:
                        xs = xt2p.tile([128, 8, 2 * CH], f16, tag="xs2", name="xs2")
                        nc.sync.dma_start(
                            xs[:], xT_v[:, :, bass.ts(2 * bb + lc // 2, 2 * CH)]
                            .rearrange("dt p t -> p dt t"))
                        xs2[(bb, lc // 2)] = xs

                def s_a(w_s, dest, nm):
                    def f():
                        ps = ovap.tile([128, CH], f32, tag="ova", name="psA")
                        for dt_i in range(8):
                            nc.tensor.matmul(
                                ps[:], w_s[:, bass.ts(dt_i, 128)],
                                xs2[(bb, lc // 2)][:, dt_i, xoff:xoff + CH],
                                start=(dt_i == 0), stop=(dt_i == 7))
                        nc.vector.tensor_copy(dest[:, csl], ps[:])
                    return f

                def s_sq(j, src_t):
                    def f():
                        sq = sq2p.tile([128, CH], f16, tag="sq2", name="sq2")
                        nc.vector.tensor_mul(sq[:], src_t[:, csl], src_t[:, csl])
                        st = ovbp.tile([2, CH], f32, tag="ovb", name="stp")
                        nc.tensor.matmul(st[:], sel_s[:], sq[:], start=True, stop=True)
                        nc.vector.tensor_copy(stqk[:, bass.ts(j, CH)], st[:])
                    return f

                def s_al():
                    rv = st2p.tile([2, 2 * CH], f32, tag="rv2", name="rv2")
                    nc.vector.reciprocal_approx_fast(rv[:], stqk[:])
                    nc.scalar.activation(alph[:], rv[:],
                                         mybir.ActivationFunctionType.Sqrt,
                                         scale=0.125)

                def s_r1(j, src_t):
                    def f():
                        sps = ovbp.tile([128, CH], f32, tag="ovb", name="spsp")
                        nc.tensor.matmul(sps[:], wsel_s[:], alph[:, bass.ts(j, CH)],
                                         start=True, stop=True)
                        qs = tm2p.tile([128, CH], f16, tag="qs2", name="qs2")
                        nc.vector.tensor_mul(qs[:], src_t[:, csl], sps[:])
                        sts.append(qs)
                    return f

                def s_r2(dest):
                    def f():
                        qs = sts[-1]
                        yp = ovbp.tile([128, CH], f32, tag="ovb", name="ypp")
                        nc.tensor.matmul(yp[:], perm_s[:], qs[:], start=True, stop=True)
                        t1 = tm2p.tile([128, CH], f16, tag="t12", name="t12")
                        nc.vector.tensor_mul(t1[:], qs[:], cos_t[:, cs])
                        t2 = tm2p.tile([128, CH], f16, tag="t22", name="t22")
                        nc.vector.tensor_mul(t2[:], yp[:], sin_t[:, cs])
                        nc.vector.tensor_add(dest[:, cs], t1[:], t2[:])
                    return f

                def s_v():
                    pt = ovap.tile([128, 4, 128], f16, tag="ova", name="ptp")
                    for jj in range(4):
                        tt = lc * 4 + jj
                        nc.tensor.transpose(pt[:, jj, :], v01b[:, bass.ts(tt, 128)],
                                            iden_s[:])
                    nc.vector.tensor_copy(vtok[:, bass.ts(c, 4), 0:E], pt[:, :, 0:E])
                    nc.vector.tensor_copy(vtok[:, bass.ts(c, 4), E + 1:2 * E + 1],
                                          pt[:, :, E:2 * E])

                return [s_x,
                        s_a(wq_s, q01b, "q"), s_a(wk_s, k01b, "k"),
                        s_a(wv_s, v01b, "v"),
                        s_sq(0, q01b), s_sq(1, k01b), s_al,
                        s_r1(0, q01b), s_r2(qhat),
                        s_r1(1, k01b), s_r2(khat),
                        s_v]

            groups = [(b, qc, kt) for b in range(B) for qc in range(NQC)
                      for kt in range(KT)]

            PROBE_SC = bool(os.environ.get("KPROBE_SCHALF"))
            PROBE_EX = bool(os.environ.get("KPROBE_EXPHALF"))

            def make_group(G):
                b, qc, kt = G
                qs_ = slice(b * S + qc * QC, b * S + (qc + 1) * QC)
                ks = slice(b * S + kt * 128, b * S + (kt + 1) * 128)
                sc = scp.tile([128, 2 * QC], f32, tag="sc", name="sc")
                nc.tensor.matmul(sc[:, 0:QC], khat[0:E, ks], qhat[0:E, qs_],
                                 start=True, stop=True)
                if not PROBE_SC:
                    nc.tensor.matmul(sc[:, QC:2 * QC], khat[E:128, ks],
                                     qhat[E:128, qs_], start=True, stop=True)
                return sc

            def do_exp(sc, G):
                ex = expp.tile([128, 2 * QC], bf16, tag="ex", name="ex")
                if PROBE_EX:
                    nc.scalar.activation(ex[:, 0:QC], sc[:, 0:QC],
                                         mybir.ActivationFunctionType.Exp,
                                         bias=nl64_s[:])
                else:
                    nc.scalar.activation(ex[:], sc[:],
                                         mybir.ActivationFunctionType.Exp,
                                         bias=nl64_s[:])
                return ex

            def emit_of(b, qc):
                blk = b * NQC + qc
                dsbs, ofs = [], []
                for h in range(2):
                    oa = oa_cur[h]
                    dsb = ofp.tile([1, QC], f32, tag="dsb", name="dsb")
                    nc.vector.tensor_scalar_mul(dsb[:], oa[E:E + 1, :], 0.015625)
                    of = ofp.tile([E + 1, QC], f16, tag="of", name="of")
                    nc.vector.tensor_scalar_mul(of[0:E, :], oa[0:E, :], 0.015625)
                    dsbs.append(dsb)
                    ofs.append(of)
                for h in range(2):
                    drf = ofp.tile([1, QC], f32, tag="drf", name="drf")
                    nc.vector.reciprocal_approx_fast(drf[:], dsbs[h][:])
                    nc.vector.tensor_copy(ofs[h][E:E + 1, :], drf[:])
                    r0 = blk * BLK + h * (E + 1)
                    nc.sync.dma_start(osh_d.ap()[r0:r0 + E + 1, :], ofs[h][:])

            def do_o(ex, G):
                b, qc, kt = G
                if kt == 0:
                    oa_cur[0] = oap.tile([E + 1, QC], f32, tag="oa0", name="oa0")
                    oa_cur[1] = oap.tile([E + 1, QC], f32, tag="oa1", name="oa1")
                tt = b * KT + kt
                for h in range(2):
                    nc.tensor.matmul(
                        oa_cur[h][:],
                        vtok[:, tt, h * (E + 1):(h + 1) * (E + 1)],
                        ex[:, h * QC:(h + 1) * QC],
                        start=(kt == 0), stop=(kt == KT - 1))
                if kt == KT - 1:
                    emit_of(b, qc)

            def mk_dsteps():
                # phase D of the just-A2A'd rep, as closures paced into the
                # NEXT rep's C loop (or run directly as the final tail)
                gn = gdp.tile([128, 8, TB], f16, tag="gn", name="gn")
                gts, drrs = [], []

                def d_dma():
                    for si in range(8):
                        g_t = dnp.tile([128, TB], f16, tag="g", name="g")
                        nc.sync.dma_start(g_t[0:E, :], ga3[si, 0:E, :])
                        nc.sync.dma_start(g_t[E:128, :], ga3[si, E + 1:2 * E + 1, :])
                        drr = dnp.tile([2, TB], f16, tag="drr", name="drr")
                        nc.sync.dma_start(
                            drr[:],
                            ga3[si].rearrange("(h r) t -> h r t", r=E + 1)[:, E, :])
                        gts.append(g_t)
                        drrs.append(drr)

                def d_norm(s0):
                    def f():
                        for si in range(s0, s0 + 4):
                            bc = ovbp.tile([128, TB], f32, tag="ovb", name="bc")
                            nc.tensor.matmul(bc[:], ones2_s[:], drrs[si][:],
                                             start=True, stop=True)
                            nc.vector.tensor_mul(gn[:, si, :], gts[si][:], bc[:])
                    return f

                def d_out(do):
                    def f():
                        po = ovap.tile([128, TB], f32, tag="ova", name="po")
                        for dt_i in range(8):
                            nc.tensor.matmul(po[:], wo_s[:, dt_i, bass.ts(do, 128)],
                                             gn[:, dt_i, :],
                                             start=(dt_i == 0), stop=(dt_i == 7))
                        ob = osb2p.tile([128, TB], f32, tag="ob", name="ob")
                        nc.vector.tensor_copy(ob[:], po[:])
                        nc.sync.dma_start(out_d.ap()[bass.ts(do, 128), :], ob[:])
                    return f

                return [d_dma, d_norm(0), d_norm(4)] + [d_out(do) for do in range(8)]

            # D-step pacing slots within the next rep's C loop (after the
            # AllToAll has had time to complete)
            DSLOTS = [8, 18, 21, 68, 72, 76, 80, 84, 88, 92, 96]

            d_pend = None
            for _rep in range(reps):
                steps1 = []
                for lc in range(NCH // 2):
                    steps1.extend(mk_steps(1, lc))
                steps0 = []
                if _rep + 1 < reps:
                    for lc in range(NCH // 2):
                        steps0.extend(mk_steps(0, lc))
                dsteps = d_pend or []
                dmap = {s: i for i, s in enumerate(DSLOTS[:len(dsteps)])}

                # AV runs OLAG groups behind exp: av(g-OLAG)'s ex dependency
                # is long satisfied when it reaches the head of the in-order
                # PE queue, so PE never head-blocks waiting on ACT. OLAG=2
                # needs expp bufs=3 (ex alive from exp(g) to av(g+2)).
                OLAG = int(os.environ.get("KOLAG", "2"))
                half = len(groups) // 2
                done1 = done0 = 0
                sc_cur = make_group(groups[0])
                opq = []
                for idx, G in enumerate(groups):
                    ex = do_exp(sc_cur, G)
                    if len(opq) >= OLAG:
                        do_o(*opq.pop(0))
                    if idx + 1 < len(groups):
                        sc_cur = make_group(groups[idx + 1])
                    opq.append((ex, G))
                    if idx in dmap:
                        dsteps[dmap[idx]]()
                    if idx < half:
                        want = len(steps1) * (idx + 1) // half
                        while done1 < want:
                            steps1[done1]()
                            done1 += 1
                    elif steps0:
                        want = len(steps0) * (idx + 1 - half) // half
                        while done0 < want:
                            steps0[done0]()
                            done0 += 1
                while opq:
                    do_o(*opq.pop(0))

                if DBG:
                    nc.sync.dma_start(dbg["dbg_qhat"].ap(), qhat[:])
                    nc.sync.dma_start(dbg["dbg_khat"].ap(), khat[:])
                    nc.sync.dma_start(dbg["dbg_vtok"].ap(),
                                      vtok[:].rearrange("p a b -> p (a b)"))
                    nc.sync.dma_start(dbg["dbg_osh"].ap(), osh_d.ap())

                if not os.environ.get("KNO_CC"):
                    nc.gpsimd.collective_compute(
                        "AllToAll", mybir.AluOpType.bypass,
                        replica_groups=[list(range(NC))],
                        ins=[osh_d.ap()], outs=[oga_d.ap()],
                    )
                d_pend = mk_dsteps()

            # final rep's D tail: bridge the AllToAll with warm-keeping
            # matmuls, then run the D steps directly
            if not os.environ.get("KNO_JUNK"):
                for _w in range(30):
                    wp = scp.tile([128, 2 * QC], f32, tag="sc", name="wp")
                    nc.tensor.matmul(wp[:, 0:TB], perm_s[:], qhat[:, 0:TB],
                                     start=True, stop=True)
            for f in d_pend:
                f()
    nc.compile()
    return nc


def make_inputs(x, position, w_qkv, w_out, norm_w, s=None):
    """Build the 8 per-core input dicts from full inputs."""
    s = s or x.shape[1]
    assert s == S
    t = x.shape[0] * s
    xT = np.ascontiguousarray(x.reshape(t, D).T).astype(np.float16)
    cos = position[0]   # [s, E]
    sin = position[1]
    cosT1 = np.ascontiguousarray(cos.T)          # [E, s]
    sinT1 = np.ascontiguousarray(sin.T)
    cosT = np.tile(np.concatenate([cosT1, cosT1], 0), (1, x.shape[0]))  # [128, t]
    sinT = np.tile(np.concatenate([sinT1, sinT1], 0), (1, x.shape[0]))

    sel = np.zeros((128, 2), np.float16)
    sel[0:E, 0] = 1.0 / E
    sel[E:128, 1] = 1.0 / E
    wsel = np.zeros((2, 128), np.float16)
    wsel[0, 0:E] = norm_w.astype(np.float16)
    wsel[1, E:128] = norm_w.astype(np.float16)
    # rope: y = P t ;  y[i] = -t[2i+1] (i<32), y[32+i] = t[2i]
    P = np.zeros((E, E), np.float16)
    for i in range(E // 2):
        P[i, 2 * i + 1] = -1.0
        P[E // 2 + i, 2 * i] = 1.0
    Pb = np.zeros((128, 128), np.float16)
    Pb[0:E, 0:E] = P
    Pb[E:128, E:128] = P
    permT = np.ascontiguousarray(Pb.T)
    iden = np.eye(128, dtype=np.float16)
    ones2 = np.zeros((2, 128), np.float16)
    ones2[0, 0:E] = 1.0
    ones2[1, E:128] = 1.0
    woT = np.ascontiguousarray(w_out.T).astype(np.float16)

    w3 = w_qkv.reshape(H, 3, E, D)
    in_maps = []
    for c in range(NC):
        h0, h1 = HPC * c, HPC * c + 1
        wqT = np.ascontiguousarray(
            np.concatenate([w3[h0, 0], w3[h1, 0]], 0).T).astype(np.float16)
        wkT = np.ascontiguousarray(
            np.concatenate([w3[h0, 1], w3[h1, 1]], 0).T).astype(np.float16)
        wvT = np.ascontiguousarray(
            np.concatenate([w3[h0, 2], w3[h1, 2]], 0).T).astype(np.float16)
        in_maps.append({
            "xT": xT, "wqT": wqT, "wkT": wkT, "wvT": wvT, "woT": woT,
            "cosT": cosT.astype(np.float16), "sinT": sinT.astype(np.float16),
            "sel": sel, "wsel": wsel, "permT": permT, "iden": iden,
            "ones2": ones2,
            "onecol": np.ones((128, t // 128), np.float16),
            "nln64": np.full((128, 1), -LNC, np.float32),
        })
    return in_maps


def assemble(results, s=None):
    s = s or S
    t = B * s
    tb = t // NC
    out = np.empty((t, D), np.float32)
    for c in range(NC):
        out[c * tb:(c + 1) * tb, :] = results[c]["out_t"].T
    return out.reshape(B, s, D)


_NC_CACHE = {}


def kernel(x, position, w_qkv, w_out, norm_w, heads):
    x = np.asarray(x, np.float32)
    position = np.asarray(position, np.float32)
    w_qkv = np.asarray(w_qkv, np.float32)
    w_out = np.asarray(w_out, np.float32)
    norm_w = np.asarray(norm_w, np.float32)
    s = x.shape[1]
    if s not in _NC_CACHE:
        _NC_CACHE[s] = build_nc(s)
    nc = _NC_CACHE[s]
    in_maps = make_inputs(x, position, w_qkv, w_out, norm_w, s=s)
    res = run_bass_kernel_spmd(nc, in_maps, list(range(NC)))
    return assemble(res.results, s=s)



# revision 53
# speedup vs baseline: 3.7019x; 3.7019x over previous
"""Trainium2 Bass kernel for nn_Attention_85710367359111.

Full (unsharded) numpy inputs in, full output out. Tensor-parallel over
heads (16 heads / 8 cores = 2 heads per core), on-device AllToAll to
re-shard from head-parallel to token-parallel before the output
projection, host-side concat of the 8 token blocks.

Key optimizations over the fp32 baseline (~440us/rep -> ~295us/rep):
- fp16 matmul operands everywhere (PE 1-pass vs fp32 4-pass), bf16 for
  exp outputs (range safety; max score ~19 overflows fp16 exp).
- The ACT (scalar) engine is the critical resource: exp of all S^2
  scores is ~147us minimum. Everything else is arranged around keeping
  its queue saturated: a flat two-stage software pipeline over all
  (batch, qchunk, kt) score groups -- while ACT runs exp(g), the PE
  runs o(g-1) and scores(g+1); no pipeline resets at qc boundaries.
- Batch-1 qkv/rms/rope/transpose work is issued in paced substeps
  interleaved into batch-0's attention loop, hiding under the ACT wall
  (2 PSUM overlay banks; scores 4 banks double-buffered + o-accum 2).
- Softmax normalization happens after the AllToAll: the denominator
  RECIPROCALS ride the collective as row 64/129 of each 130-row block,
  so phase D is one broadcast matmul + multiply per source core.
- exp computes exp(s - ln1024) and o is shipped scaled by 1/64
  (compensated by 64/d) to stay inside fp16 range end to end.
- Dummy matmuls bridge the AllToAll window so the PE HAM clock gate
  does not drop to the cold 4/8 rate before the output projection.
- Large merged DMAs (one per weight tensor / x strip) keep the Sync
  queue short; the first x strip is issued before cos/sin so phase A
  starts immediately.

Per-core pipeline (feature-major [feat_on_partitions, tokens]):
  A  qkv^T = w_qkv_slice @ x^T            (PE, fp16 -> fp32 PSUM)
  B  RMS stats via selector-matmul -> reciprocal(DVE) -> sqrt(ACT),
     scale via rank-2 broadcast matmul, RoPE via +-1 permutation
     matmul + cos/sin elementwise (DVE)
  B' v^T -> token-major via PE transposes (with ones columns)
  C  scores^T = k^T.T @ q^T per head (row-tiled, both heads concurrent
     on PE) -> exp (ACT) -> o_aug = [v|1].T @ ex accumulated over kt
     in PSUM -> fp16 -> DRAM
  D  AllToAll of [o_unnorm/64; 64/denom] blocks, broadcast matmul +
     multiply normalize, out^T = w_out^T.T @ gathered (row-parallel,
     no all-reduce needed)
"""

import math
import os

import numpy as np

import concourse.bacc as bacc
import concourse.bass as bass
import concourse.tile as tile
from concourse import mybir
from concourse.bass_utils import run_bass_kernel_spmd

# ---------------------------------------------------------------- config

B, S, D, H, E = 2, 2048, 1024, 16, 64
NC = 8                      # cores
HPC = H // NC               # heads per core = 2
DL = HPC * E                # local d slice = 128
T = B * S                   # total tokens
TB = T // NC                # tokens per A2A block = 512
KT = S // 128               # k tiles per batch = 16
NTT = T // 128              # total tok tiles = 32
QC = 512                    # q chunk
NQC = S // QC               # q chunks per batch = 4
CH = 512                    # phase A/B token chunk
NCH = T // CH               # chunks total = 8
BLK = 2 * (E + 1)           # osh rows per token block = 130
EPS = float(np.finfo(np.float32).eps)
LNC = float(math.log(1024.0))

f16 = mybir.dt.float16
bf16 = mybir.dt.bfloat16
f32 = mybir.dt.float32

# kt group pattern per qc: strictly alternating PSUM tags (big slot holds
# 1-2 kt, small slot 1 kt) so consecutive exp groups never share a score
# tile -- including across qc boundaries (12 groups, b/s/b/s/...)
KT_GROUPS = [(0, 2, "b"), (2, 1, "s"), (3, 2, "b"), (5, 1, "s"),
             (6, 2, "b"), (8, 1, "s"), (9, 2, "b"), (11, 1, "s"),
             (12, 1, "b"), (13, 1, "s"), (14, 1, "b"), (15, 1, "s")]
assert sum(g[1] for g in KT_GROUPS) == KT


def build_nc(s=S, reps=1):
    assert s == S, "kernel hardcoded for S=2048"
    nc = bacc.Bacc("TRN2", target_bir_lowering=False, debug=False, num_devices=NC)

    # ------------- DRAM I/O
    xT_d = nc.dram_tensor("xT", [D, T], f16, kind="ExternalInput")
    wq_d = nc.dram_tensor("wqT", [D, DL], f16, kind="ExternalInput")
    wk_d = nc.dram_tensor("wkT", [D, DL], f16, kind="ExternalInput")
    wv_d = nc.dram_tensor("wvT", [D, DL], f16, kind="ExternalInput")
    wo_d = nc.dram_tensor("woT", [D, D], f16, kind="ExternalInput")
    cos_d = nc.dram_tensor("cosT", [128, T], f16, kind="ExternalInput")
    sin_d = nc.dram_tensor("sinT", [128, T], f16, kind="ExternalInput")
    sel_d = nc.dram_tensor("sel", [128, 2], f16, kind="ExternalInput")
    wsel_d = nc.dram_tensor("wsel", [2, 128], f16, kind="ExternalInput")
    perm_d = nc.dram_tensor("permT", [128, 128], f16, kind="ExternalInput")
    iden_d = nc.dram_tensor("iden", [128, 128], f16, kind="ExternalInput")
    ones2_d = nc.dram_tensor("ones2", [2, 128], f16, kind="ExternalInput")
    onec_d = nc.dram_tensor("onecol", [128, NTT], f16, kind="ExternalInput")
    nl64_d = nc.dram_tensor("nln64", [128, 1], f32, kind="ExternalInput")

    osh_d = nc.dram_tensor("o_shard", [NC * BLK, TB], f16)
    oga_d = nc.dram_tensor("o_gath", [NC * BLK, TB], f16)
    out_d = nc.dram_tensor("out_t", [D, TB], f32, kind="ExternalOutput")
    DBG = bool(os.environ.get("KDEBUG"))
    if DBG:
        dbg = {}
        for nm, shp, dt_ in [("dbg_q01", [128, T], f32), ("dbg_qhat", [128, T], f16),
                             ("dbg_khat", [128, T], f16), ("dbg_v01", [128, T], f16),
                             ("dbg_vtok", [128, NTT * BLK], f16),
                             ("dbg_osh", [NC * BLK, TB], f16),
                             ("dbg_oga", [NC * BLK, TB], f16),
                             ("dbg_gn", [128, 8 * TB], f16),
                             ("dbg_sc", [128, 4 * QC], f32),
                             ("dbg_ex", [128, 4 * QC], bf16)]:
            dbg[nm] = nc.dram_tensor(nm, shp, dt_, kind="ExternalOutput")

    xT_v = xT_d.ap().rearrange("(dt p) t -> dt p t", p=128)     # [8,128,T]
    wo_v = wo_d.ap().rearrange("(dt p) o -> dt p o", p=128)     # [8,128,D]

    from contextlib import ExitStack

    with tile.TileContext(nc) as tc, ExitStack() as ctx:
      if True:
          if True:
            pers = ctx.enter_context(tc.tile_pool(name="pers", bufs=1))
            qhat = pers.tile([128, T], f16, tag="qhat")
            khat = pers.tile([128, T], f16, tag="khat")
            vtok = pers.tile([128, NTT, BLK], f16, tag="vtok")
            wq_s = pers.tile([128, D], f16, tag="wq")
            wk_s = pers.tile([128, D], f16, tag="wk")
            wv_s = pers.tile([128, D], f16, tag="wv")
            wo_s = pers.tile([128, 8, D], f16, tag="wo")
            sel_s = pers.tile([128, 2], f16, tag="sel")
            wsel_s = pers.tile([2, 128], f16, tag="wsel")
            perm_s = pers.tile([128, 128], f16, tag="perm")
            iden_s = pers.tile([128, 128], f16, tag="iden")
            ones2_s = pers.tile([2, 128], f16, tag="ones2")
            nl64_s = pers.tile([128, 1], f32, tag="nl64")
            q01x = [pers.tile([128, S], f32, tag=f"q01_{bb}", name=f"q01_{bb}")
                    for bb in range(2)]
            k01x = [pers.tile([128, S], f32, tag=f"k01_{bb}", name=f"k01_{bb}")
                    for bb in range(2)]
            v01x = [pers.tile([128, S], f16, tag=f"v01_{bb}", name=f"v01_{bb}")
                    for bb in range(2)]

            for w_s, w_d in ((wq_s, wq_d), (wk_s, wk_d), (wv_s, wv_d)):
                nc.sync.dma_start(
                    w_s[:].rearrange("p (dt c) -> p dt c", dt=8),
                    w_d.ap().rearrange("(dt p) c -> p dt c", p=128))
            nc.sync.dma_start(sel_s[:], sel_d.ap())
            nc.sync.dma_start(wsel_s[:], wsel_d.ap())
            nc.sync.dma_start(perm_s[:], perm_d.ap())
            nc.sync.dma_start(iden_s[:], iden_d.ap())
            nc.sync.dma_start(ones2_s[:], ones2_d.ap())
            nc.sync.dma_start(nl64_s[:], nl64_d.ap())
            # ones columns of vtok (col 64 for head0, col 129 for head1)
            nc.sync.dma_start(vtok[:, :, E], onec_d.ap())
            nc.sync.dma_start(vtok[:, :, 2 * E + 1], onec_d.ap())

            cos_t = pers.tile([128, T], f16, tag="cos")
            sin_t = pers.tile([128, T], f16, tag="sin")

            # ---------------- phases A+B+B', chunk-pipelined
            with tc.tile_pool(name="xt", bufs=2) as xtp, \
                 tc.tile_pool(name="sq", bufs=3) as sqp, \
                 tc.tile_pool(name="st", bufs=3) as stp, \
                 tc.tile_pool(name="psa", bufs=2, space="PSUM") as psa, \
                 tc.tile_pool(name="pst", bufs=1, space="PSUM") as pst, \
                 tc.tile_pool(name="pss", bufs=2, space="PSUM") as pss, \
                 tc.tile_pool(name="psy", bufs=1, space="PSUM") as psy, \
                 tc.tile_pool(name="psv", bufs=1, space="PSUM") as psv, \
                 tc.tile_pool(name="tmp", bufs=3) as tmpp:
                q01, k01, v01 = q01x[0], k01x[0], v01x[0]

                def load_strip(si):
                    xs = xtp.tile([128, 8, 2 * CH], f16, tag="xs")
                    nc.sync.dma_start(
                        xs[:],
                        xT_v[:, :, bass.ts(si, 2 * CH)].rearrange("dt p t -> p dt t"))
                    return xs

                # first x strip before cos/sin so phase A starts ASAP
                xs_cur = load_strip(0)
                nc.sync.dma_start(cos_t[:], cos_d.ap())
                nc.sync.dma_start(sin_t[:], sin_d.ap())

                for c in range(NCH // 2):
                    cs = bass.ts(c, CH)
                    # ---- A: qkv^T = W @ x^T; x loaded in 1024-tok strips,
                    # one DMA per strip for all 8 contraction tiles
                    if c % 2 == 0 and c > 0:
                        xs_cur = load_strip(c // 2)
                    xoff = (c % 2) * CH
                    for w_s, dest in ((wq_s, q01), (wk_s, k01), (wv_s, v01)):
                        ps = psa.tile([128, CH], f32, tag="ps")
                        for dt_i in range(8):
                            nc.tensor.matmul(
                                ps[:], w_s[:, bass.ts(dt_i, 128)],
                                xs_cur[:, dt_i, xoff:xoff + CH],
                                start=(dt_i == 0), stop=(dt_i == 7),
                            )
                        nc.scalar.copy(dest[:, cs], ps[:])

                    # ---- B: rms stats -> alpha -> scale -> rope (q,k)
                    st = pst.tile([2, 2 * CH], f32, tag="st")
                    for j, src_t in enumerate((q01, k01)):
                        sq = sqp.tile([128, CH], f16, tag="sq")
                        nc.scalar.activation(sq[:], src_t[:, cs],
                                             mybir.ActivationFunctionType.Square)
                        nc.tensor.matmul(st[:, bass.ts(j, CH)], sel_s[:], sq[:],
                                         start=True, stop=True)
                    # alpha/sqrt(8) = sqrt((1/8) * 1/mean)
                    rv = stp.tile([2, 2 * CH], f32, tag="rv")
                    nc.vector.reciprocal_approx_fast(rv[:], st[:])
                    alpha = stp.tile([2, 2 * CH], f16, tag="alpha")
                    nc.scalar.activation(alpha[:], rv[:],
                                         mybir.ActivationFunctionType.Sqrt,
                                         scale=0.125)
                    for j, (src_t, dest) in enumerate(((q01, qhat), (k01, khat))):
                        sps = pss.tile([128, CH], f32, tag="sps")
                        nc.tensor.matmul(sps[:], wsel_s[:], alpha[:, bass.ts(j, CH)],
                                         start=True, stop=True)
                        qs = tmpp.tile([128, CH], f16, tag="qs")
                        nc.vector.tensor_mul(qs[:], src_t[:, cs], sps[:])
                        yp = psy.tile([128, CH], f32, tag="yp")
                        nc.tensor.matmul(yp[:], perm_s[:], qs[:], start=True, stop=True)
                        t1 = tmpp.tile([128, CH], f16, tag="t1")
                        nc.vector.tensor_mul(t1[:], qs[:], cos_t[:, cs])
                        t2 = tmpp.tile([128, CH], f16, tag="t2")
                        nc.vector.tensor_mul(t2[:], yp[:], sin_t[:, cs])
                        nc.vector.tensor_add(dest[:, cs], t1[:], t2[:])

                    # ---- B': v -> token-major (+ ones cols already set)
                    g = c  # chunk == group of 4 token tiles
                    pt = psv.tile([128, 4, 128], f16, tag="pt")
                    for jj in range(4):
                        tt = g * 4 + jj
                        nc.tensor.transpose(pt[:, jj, :], v01[:, bass.ts(tt, 128)], iden_s[:])
                    nc.vector.tensor_copy(vtok[:, bass.ts(g, 4), 0:E], pt[:, :, 0:E])
                    nc.vector.tensor_copy(vtok[:, bass.ts(g, 4), E + 1:2 * E + 1],
                                          pt[:, :, E:2 * E])


            # wo loads late -- only phase D needs it
            nc.sync.dma_start(wo_s[:], wo_d.ap().rearrange("(dt p) o -> p dt o", p=128))

            # ------- persistent pools for C / overlay / D (all reps)
            scp = ctx.enter_context(tc.tile_pool(name="scp", bufs=2, space="PSUM"))
            oap = ctx.enter_context(tc.tile_pool(name="oap", bufs=1, space="PSUM"))
            ovap = ctx.enter_context(tc.tile_pool(name="ova", bufs=1, space="PSUM"))
            ovbp = ctx.enter_context(tc.tile_pool(name="ovb", bufs=1, space="PSUM"))
            xt2p = ctx.enter_context(tc.tile_pool(name="xt2", bufs=2))
            sq2p = ctx.enter_context(tc.tile_pool(name="sq2", bufs=2))
            st2p = ctx.enter_context(tc.tile_pool(name="st2", bufs=2))
            tm2p = ctx.enter_context(tc.tile_pool(name="tm2", bufs=2))
            expp = ctx.enter_context(tc.tile_pool(name="exp", bufs=3))
            ofp = ctx.enter_context(tc.tile_pool(name="ofp", bufs=2))
            gdp = ctx.enter_context(tc.tile_pool(name="gd", bufs=1))
            dnp = ctx.enter_context(tc.tile_pool(name="dnp", bufs=8))
            osb2p = ctx.enter_context(tc.tile_pool(name="osb2", bufs=2))
            ga = osh_d.ap() if os.environ.get("KNO_CC") else oga_d.ap()
            ga3 = ga.rearrange("(s r) t -> s r t", r=BLK)   # [8, 130, TB]
            xs2 = {}
            oa_cur = {}

            def mk_steps(bb, lc):
                c = bb * (NCH // 2) + lc    # global chunk index
                cs = bass.ts(c, CH)
                csl = bass.ts(lc, CH)
                xoff = (lc % 2) * CH
                q01b, k01b, v01b = q01x[bb], k01x[bb], v01x[bb]
                stqk = st2p.tile([2, 2 * CH], f32, tag="stqk", name="stqk")
                alph = st2p.tile([2, 2 * CH], f16, tag="alph", name="alph")
                sts = []

                def s_x():
                    if lc % 2 == 0:
                        xs = xt2p.tile([128, 8, 2 * CH], f16, tag="xs2", name="xs2")
                        nc.sync.dma_start(
                            xs[:], xT_v[:, :, bass.ts(2 * bb + lc // 2, 2 * CH)]
                            .rearrange("dt p t -> p dt t"))
                        xs2[(bb, lc // 2)] = xs

                def s_a(w_s, dest, nm):
                    def f():
                        ps = ovap.tile([128, CH], f32, tag="ova", name="psA")
                        for dt_i in range(8):
                            nc.tensor.matmul(
                                ps[:], w_s[:, bass.ts(dt_i, 128)],
                                xs2[(bb, lc // 2)][:, dt_i, xoff:xoff + CH],
                                start=(dt_i == 0), stop=(dt_i == 7))
                        nc.vector.tensor_copy(dest[:, csl], ps[:])
                    return f

                def s_sq(j, src_t):
                    def f():
                        sq = sq2p.tile([128, CH], f16, tag="sq2", name="sq2")
                        nc.vector.tensor_mul(sq[:], src_t[:, csl], src_t[:, csl])
                        st = ovbp.tile([2, CH], f32, tag="ovb", name="stp")
                        nc.tensor.matmul(st[:], sel_s[:], sq[:], start=True, stop=True)
                        nc.vector.tensor_copy(stqk[:, bass.ts(j, CH)], st[:])
                    return f

                def s_al():
                    rv = st2p.tile([2, 2 * CH], f32, tag="rv2", name="rv2")
                    nc.vector.reciprocal_approx_fast(rv[:], stqk[:])
                    nc.scalar.activation(alph[:], rv[:],
                                         mybir.ActivationFunctionType.Sqrt,
                                         scale=0.125)

                def s_r1(j, src_t):
                    def f():
                        sps = ovbp.tile([128, CH], f32, tag="ovb", name="spsp")
                        nc.tensor.matmul(sps[:], wsel_s[:], alph[:, bass.ts(j, CH)],
                                         start=True, stop=True)
                        qs = tm2p.tile([128, CH], f16, tag="qs2", name="qs2")
                        nc.vector.tensor_mul(qs[:], src_t[:, csl], sps[:])
                        sts.append(qs)
                    return f

                def s_r2(dest):
                    def f():
                        qs = sts[-1]
                        yp = ovbp.tile([128, CH], f32, tag="ovb", name="ypp")
                        nc.tensor.matmul(yp[:], perm_s[:], qs[:], start=True, stop=True)
                        t1 = tm2p.tile([128, CH], f16, tag="t12", name="t12")
                        nc.vector.tensor_mul(t1[:], qs[:], cos_t[:, cs])
                        t2 = tm2p.tile([128, CH], f16, tag="t22", name="t22")
                        nc.vector.tensor_mul(t2[:], yp[:], sin_t[:, cs])
                        nc.vector.tensor_add(dest[:, cs], t1[:], t2[:])
                    return f

                def s_v():
                    pt = ovap.tile([128, 4, 128], f16, tag="ova", name="ptp")
                    for jj in range(4):
                        tt = lc * 4 + jj
                        nc.tensor.transpose(pt[:, jj, :], v01b[:, bass.ts(tt, 128)],
                                            iden_s[:])
                    nc.vector.tensor_copy(vtok[:, bass.ts(c, 4), 0:E], pt[:, :, 0:E])
                    nc.vector.tensor_copy(vtok[:, bass.ts(c, 4), E + 1:2 * E + 1],
                                          pt[:, :, E:2 * E])

                return [s_x,
                        s_a(wq_s, q01b, "q"), s_a(wk_s, k01b, "k"),
                        s_a(wv_s, v01b, "v"),
                        s_sq(0, q01b), s_sq(1, k01b), s_al,
                        s_r1(0, q01b), s_r2(qhat),
                        s_r1(1, k01b), s_r2(khat),
                        s_v]

            groups = [(b, qc, kt) for b in range(B) for qc in range(NQC)
                      for kt in range(KT)]

            PROBE_SC = bool(os.environ.get("KPROBE_SCHALF"))
            PROBE_EX = bool(os.environ.get("KPROBE_EXPHALF"))

            def make_group(G):
                b, qc, kt = G
                qs_ = slice(b * S + qc * QC, b * S + (qc + 1) * QC)
                ks = slice(b * S + kt * 128, b * S + (kt + 1) * 128)
                sc = scp.tile([128, 2 * QC], f32, tag="sc", name="sc")
                nc.tensor.matmul(sc[:, 0:QC], khat[0:E, ks], qhat[0:E, qs_],
                                 start=True, stop=True)
                if not PROBE_SC:
                    nc.tensor.matmul(sc[:, QC:2 * QC], khat[E:128, ks],
                                     qhat[E:128, qs_], start=True, stop=True)
                return sc

            def do_exp(sc, G):
                ex = expp.tile([128, 2 * QC], bf16, tag="ex", name="ex")
                if PROBE_EX:
                    nc.scalar.activation(ex[:, 0:QC], sc[:, 0:QC],
                                         mybir.ActivationFunctionType.Exp,
                                         bias=nl64_s[:])
                else:
                    nc.scalar.activation(ex[:], sc[:],
                                         mybir.ActivationFunctionType.Exp,
                                         bias=nl64_s[:])
                return ex

            def emit_of(b, qc):
                blk = b * NQC + qc
                dsbs, ofs = [], []
                for h in range(2):
                    oa = oa_cur[h]
                    dsb = ofp.tile([1, QC], f32, tag="dsb", name="dsb")
                    nc.vector.tensor_scalar_mul(dsb[:], oa[E:E + 1, :], 0.015625)
                    of = ofp.tile([E + 1, QC], f16, tag="of", name="of")
                    nc.vector.tensor_scalar_mul(of[0:E, :], oa[0:E, :], 0.015625)
                    dsbs.append(dsb)
                    ofs.append(of)
                for h in range(2):
                    drf = ofp.tile([1, QC], f32, tag="drf", name="drf")
                    nc.vector.reciprocal_approx_fast(drf[:], dsbs[h][:])
                    nc.vector.tensor_copy(ofs[h][E:E + 1, :], drf[:])
                    r0 = blk * BLK + h * (E + 1)
                    nc.sync.dma_start(osh_d.ap()[r0:r0 + E + 1, :], ofs[h][:])

            def do_o(ex, G):
                b, qc, kt = G
                if kt == 0:
                    oa_cur[0] = oap.tile([E + 1, QC], f32, tag="oa0", name="oa0")
                    oa_cur[1] = oap.tile([E + 1, QC], f32, tag="oa1", name="oa1")
                tt = b * KT + kt
                for h in range(2):
                    nc.tensor.matmul(
                        oa_cur[h][:],
                        vtok[:, tt, h * (E + 1):(h + 1) * (E + 1)],
                        ex[:, h * QC:(h + 1) * QC],
                        start=(kt == 0), stop=(kt == KT - 1))
                if kt == KT - 1:
                    emit_of(b, qc)

            def mk_dsteps():
                # phase D of the just-A2A'd rep, as closures paced into the
                # NEXT rep's C loop (or run directly as the final tail)
                gn = gdp.tile([128, 8, TB], f16, tag="gn", name="gn")
                gts, drrs = [], []

                def d_dma():
                    for si in range(8):
                        g_t = dnp.tile([128, TB], f16, tag="g", name="g")
                        nc.sync.dma_start(g_t[0:E, :], ga3[si, 0:E, :])
                        nc.sync.dma_start(g_t[E:128, :], ga3[si, E + 1:2 * E + 1, :])
                        drr = dnp.tile([2, TB], f16, tag="drr", name="drr")
                        nc.sync.dma_start(
                            drr[:],
                            ga3[si].rearrange("(h r) t -> h r t", r=E + 1)[:, E, :])
                        gts.append(g_t)
                        drrs.append(drr)

                def d_norm(s0):
                    def f():
                        for si in range(s0, s0 + 4):
                            bc = ovbp.tile([128, TB], f32, tag="ovb", name="bc")
                            nc.tensor.matmul(bc[:], ones2_s[:], drrs[si][:],
                                             start=True, stop=True)
                            nc.vector.tensor_mul(gn[:, si, :], gts[si][:], bc[:])
                    return f

                def d_out(do):
                    def f():
                        po = ovap.tile([128, TB], f32, tag="ova", name="po")
                        for dt_i in range(8):
                            nc.tensor.matmul(po[:], wo_s[:, dt_i, bass.ts(do, 128)],
                                             gn[:, dt_i, :],
                                             start=(dt_i == 0), stop=(dt_i == 7))
                        ob = osb2p.tile([128, TB], f32, tag="ob", name="ob")
                        nc.vector.tensor_copy(ob[:], po[:])
                        nc.sync.dma_start(out_d.ap()[bass.ts(do, 128), :], ob[:])
                    return f

                return [d_dma, d_norm(0), d_norm(4)] + [d_out(do) for do in range(8)]

            # D-step pacing slots within the next rep's C loop (after the
            # AllToAll has had time to complete)
            DSLOTS = [8, 18, 21, 68, 72, 76, 80, 84, 88, 92, 96]

            d_pend = None
            for _rep in range(reps):
                steps1 = []
                for lc in range(NCH // 2):
                    steps1.extend(mk_steps(1, lc))
                steps0 = []
                if _rep + 1 < reps:
                    for lc in range(NCH // 2):
                        steps0.extend(mk_steps(0, lc))
                dsteps = d_pend or []
                dmap = {s: i for i, s in enumerate(DSLOTS[:len(dsteps)])}

                # AV runs OLAG groups behind exp: av(g-OLAG)'s ex dependency
                # is long satisfied when it reaches the head of the in-order
                # PE queue, so PE never head-blocks waiting on ACT. OLAG=2
                # needs expp bufs=3 (ex alive from exp(g) to av(g+2)).
                OLAG = int(os.environ.get("KOLAG", "2"))
                half = len(groups) // 2
                done1 = done0 = 0
                sc_cur = make_group(groups[0])
                opq = []
                for idx, G in enumerate(groups):
                    ex = do_exp(sc_cur, G)
                    if len(opq) >= OLAG:
                        do_o(*opq.pop(0))
                    if idx + 1 < len(groups):
                        sc_cur = make_group(groups[idx + 1])
                    opq.append((ex, G))
                    if idx in dmap:
                        dsteps[dmap[idx]]()
                    if idx < half:
                        want = len(steps1) * (idx + 1) // half
                        while done1 < want:
                            steps1[done1]()
                            done1 += 1
                    elif steps0:
                        want = len(steps0) * (idx + 1 - half) // half
                        while done0 < want:
                            steps0[done0]()
                            done0 += 1
                while opq:
                    do_o(*opq.pop(0))

                if DBG:
                    nc.sync.dma_start(dbg["dbg_qhat"].ap(), qhat[:])
                    nc.sync.dma_start(dbg["dbg_khat"].ap(), khat[:])
                    nc.sync.dma_start(dbg["dbg_vtok"].ap(),
                                      vtok[:].rearrange("p a b -> p (a b)"))
                    nc.sync.dma_start(dbg["dbg_osh"].ap(), osh_d.ap())

                if not os.environ.get("KNO_CC"):
                    nc.gpsimd.collective_compute(
                        "AllToAll", mybir.AluOpType.bypass,
                        replica_groups=[list(range(NC))],
                        ins=[osh_d.ap()], outs=[oga_d.ap()],
                    )
                d_pend = mk_dsteps()

            # final rep's D tail: bridge the AllToAll with warm-keeping
            # matmuls, then run the D steps directly
            if not os.environ.get("KNO_JUNK"):
                for _w in range(30):
                    wp = scp.tile([128, 2 * QC], f32, tag="sc", name="wp")
                    nc.tensor.matmul(wp[:, 0:TB], perm_s[:], qhat[:, 0:TB],
                                     start=True, stop=True)
            for f in d_pend:
                f()
    nc.compile()
    return nc


def make_inputs(x, position, w_qkv, w_out, norm_w, s=None):
    """Build the 8 per-core input dicts from full inputs."""
    s = s or x.shape[1]
    assert s == S
    t = x.shape[0] * s
    xT = np.ascontiguousarray(x.reshape(t, D).T).astype(np.float16)
    cos = position[0]   # [s, E]
    sin = position[1]
    cosT1 = np.ascontiguousarray(cos.T)          # [E, s]
    sinT1 = np.ascontiguousarray(sin.T)
    cosT = np.tile(np.concatenate([cosT1, cosT1], 0), (1, x.shape[0]))  # [128, t]
    sinT = np.tile(np.concatenate([sinT1, sinT1], 0), (1, x.shape[0]))

    sel = np.zeros((128, 2), np.float16)
    sel[0:E, 0] = 1.0 / E
    sel[E:128, 1] = 1.0 / E
    wsel = np.zeros((2, 128), np.float16)
    wsel[0, 0:E] = norm_w.astype(np.float16)
    wsel[1, E:128] = norm_w.astype(np.float16)
    # rope: y = P t ;  y[i] = -t[2i+1] (i<32), y[32+i] = t[2i]
    P = np.zeros((E, E), np.float16)
    for i in range(E // 2):
        P[i, 2 * i + 1] = -1.0
        P[E // 2 + i, 2 * i] = 1.0
    Pb = np.zeros((128, 128), np.float16)
    Pb[0:E, 0:E] = P
    Pb[E:128, E:128] = P
    permT = np.ascontiguousarray(Pb.T)
    iden = np.eye(128, dtype=np.float16)
    ones2 = np.zeros((2, 128), np.float16)
    ones2[0, 0:E] = 1.0
    ones2[1, E:128] = 1.0
    woT = np.ascontiguousarray(w_out.T).astype(np.float16)

    w3 = w_qkv.reshape(H, 3, E, D)
    in_maps = []
    for c in range(NC):
        h0, h1 = HPC * c, HPC * c + 1
        wqT = np.ascontiguousarray(
            np.concatenate([w3[h0, 0], w3[h1, 0]], 0).T).astype(np.float16)
        wkT = np.ascontiguousarray(
            np.concatenate([w3[h0, 1], w3[h1, 1]], 0).T).astype(np.float16)
        wvT = np.ascontiguousarray(
            np.concatenate([w3[h0, 2], w3[h1, 2]], 0).T).astype(np.float16)
        in_maps.append({
            "xT": xT, "wqT": wqT, "wkT": wkT, "wvT": wvT, "woT": woT,
            "cosT": cosT.astype(np.float16), "sinT": sinT.astype(np.float16),
            "sel": sel, "wsel": wsel, "permT": permT, "iden": iden,
            "ones2": ones2,
            "onecol": np.ones((128, t // 128), np.float16),
            "nln64": np.full((128, 1), -LNC, np.float32),
        })
    return in_maps


def assemble(results, s=None):
    s = s or S
    t = B * s
    tb = t // NC
    out = np.empty((t, D), np.float32)
    for c in range(NC):
        out[c * tb:(c + 1) * tb, :] = results[c]["out_t"].T
    return out.reshape(B, s, D)


_NC_CACHE = {}


def kernel(x, position, w_qkv, w_out, norm_w, heads):
    x = np.asarray(x, np.float32)
    position = np.asarray(position, np.float32)
    w_qkv = np.asarray(w_qkv, np.float32)
    w_out = np.asarray(w_out, np.float32)
    norm_w = np.asarray(norm_w, np.float32)
    s = x.shape[1]
    if s not in _NC_CACHE:
        _NC_CACHE[s] = build_nc(s)
    nc = _NC_CACHE[s]
    in_maps = make_inputs(x, position, w_qkv, w_out, norm_w, s=s)
    res = run_bass_kernel_spmd(nc, in_maps, list(range(NC)))
    return assemble(res.results, s=s)

